# revision 31
# baseline (speedup 1.0000x reference)
"""Trainium2 Bass kernel for nn_Interactor (attention-augmented LSTM).

Problem: B=64, T=512, L=48, DV=DS=H=512.
  per step t: proj_V = x_t W_V^T; proj_R = h W_R^T
              e = tanh(proj_R[:,None,:] + proj_S + proj_V[:,None,:])
              alpha = softmax(e @ w, axis=L); h_ts = alpha @ h_s
              gates = [x_t, h_ts] W_ih^T + h W_hh^T + b; LSTM cell update.

Sharding: data-parallel over batch: 8 cores x 8 batch rows, weights replicated.

v2 design notes (vs v1):
 - All tensor-engine operands in bf16: fp32 matmuls are emitted as 2 HW
   instructions (HI/LO split) and disable fast-weight-load; bf16 halves
   instruction count and cuts LDWEIGHTS from ~325ns to ~55ns per tile.
 - LSTM sigmoid eliminated via sigma(x) = 0.5*tanh(x/2)+0.5 (i/f/o weight rows
   pre-scaled 0.5 on host). Kernel uses only {tanh, exp} => single ACT table
   set, killing 2x ACT_TABLE_LOAD (~2.7us) per step.
 - h_ts = alpha @ h_s computed on the tensor engine: alpha lands on
   partitions via outer-product matmul expb_chunk^T (x) rec, masked to
   block-diagonal A, then h_s-stationary matmuls produce h_ts^T [D,b].
 - W_hh @ h_prev issued right after proj_R so it overlaps the attention
   chain on DVE/ACT; W_ihS @ h_ts completes the PSUM group later.
 - Gate order remapped to [i, f, o, g] so one batched tanh + one affine
   covers the pointwise nonlinearities.
"""

import numpy as np
import ml_dtypes

import concourse.bass as bass
import concourse.mybir as mybir
import concourse.tile as tile
from concourse import bacc
from concourse.bass_utils import run_bass_kernel_spmd

F32 = mybir.dt.float32
BF16 = mybir.dt.bfloat16
AF = mybir.ActivationFunctionType
ALU = mybir.AluOpType
AX = mybir.AxisListType
NP_BF16 = ml_dtypes.bfloat16

B, T_FULL, L = 64, 512, 48
DV, DS, H = 512, 512, 512
G4 = 4 * H
NCORES = 8
BLOC = B // NCORES  # 8
BL = BLOC * L       # 384
KH = H // 128       # 4 H-chunks
KM = G4 // 128      # 16 gate-row chunks
KA = BL // 128      # 3 (b,l)-chunks


def build_nc(T=T_FULL, Tc=16, debug=False, dbg_t=0):
    """Build the per-core Bass program (SPMD; same program all cores)."""
    assert T % Tc == 0
    nc = bacc.Bacc()

    # ---- DRAM I/O (per-core slices fed via in_maps) ----
    hvT = nc.declare_dram_parameter("hvT", [DV, T * BLOC], BF16, isOutput=False)
    hsT = nc.declare_dram_parameter("hsT", [DS, BL], BF16, isOutput=False)
    WS_T = nc.declare_dram_parameter("WS_T", [DS, H], BF16, isOutput=False)
    WV_T = nc.declare_dram_parameter("WV_T", [DV, H], BF16, isOutput=False)
    WihV_T = nc.declare_dram_parameter("WihV_T", [DV, G4], BF16, isOutput=False)
    WihS_T = nc.declare_dram_parameter("WihS_T", [DS, G4], BF16, isOutput=False)
    Whh_T = nc.declare_dram_parameter("Whh_T", [H, G4], BF16, isOutput=False)
    WR_T = nc.declare_dram_parameter("WR_T", [H, H], BF16, isOutput=False)
    wvec = nc.declare_dram_parameter("wvec", [H, 1], BF16, isOutput=False)
    maskD = nc.declare_dram_parameter("maskD", [BL, BLOC], F32, isOutput=False)
    identD = nc.declare_dram_parameter("identD", [128, 128], BF16, isOutput=False)
    biasRSV = nc.declare_dram_parameter("biasRSV", [128, KH], F32, isOutput=False)
    biasIH = nc.declare_dram_parameter("biasIH", [128, KM], F32, isOutput=False)
    bw = nc.declare_dram_parameter("bw", [1, 1], F32, isOutput=False)
    out_c = nc.declare_dram_parameter("out_c", [T, KH, 128, BLOC], BF16, isOutput=True)

    # ---- internal DRAM for precomputed projections ----
    GV_d = nc.dram_tensor("GV_d", [T, KM, 128, BLOC], BF16)
    PV_d = nc.dram_tensor("PV_d", [T, KH, 128, BLOC], BF16)

    if debug:
        dbg = {
            "dbg_rvt": nc.dram_tensor("dbg_rvt", [128, KH * BLOC], BF16, kind="ExternalOutput"),
            "dbg_e": nc.dram_tensor("dbg_e", [128, KH, BL], BF16, kind="ExternalOutput"),
            "dbg_expb": nc.dram_tensor("dbg_expb", [1, BL], BF16, kind="ExternalOutput"),
            "dbg_A": nc.dram_tensor("dbg_A", [128, KA, BLOC], BF16, kind="ExternalOutput"),
            "dbg_gates": nc.dram_tensor("dbg_gates", [128, KM * BLOC], F32, kind="ExternalOutput"),
            "dbg_th": nc.dram_tensor("dbg_th", [128, KM * BLOC], F32, kind="ExternalOutput"),
            "dbg_c": nc.dram_tensor("dbg_c", [128, 32], F32, kind="ExternalOutput"),
        }

    NT = T * BLOC
    NCW = min(512, NT)
    n_nc = NT // NCW

    with tile.TileContext(nc) as tc:
        with (
            tc.tile_pool(name="res", bufs=1) as res,
            tc.tile_pool(name="state", bufs=2) as state,
            tc.tile_pool(name="hout", bufs=2) as houtp,
            tc.tile_pool(name="stream", bufs=2) as stream,
            tc.tile_pool(name="work", bufs=2) as work,
        ):
            # ---------- resident loads ----------
            wr_sb = res.tile([128, KH, H], BF16, tag="wr")
            nc.sync.dma_start(out=wr_sb, in_=WR_T.rearrange("(k p) m -> p k m", p=128))
            whh_sb = res.tile([128, KH, G4], BF16, tag="whh")
            nc.sync.dma_start(out=whh_sb, in_=Whh_T.rearrange("(k p) m -> p k m", p=128))
            psg_sb = res.tile([128, KA, G4], BF16, tag="psg")
            wvec_sb = res.tile([128, KH], BF16, tag="wvec")
            nc.sync.dma_start(out=wvec_sb, in_=wvec.rearrange("(k p) o -> p (k o)", p=128))
            mask_sb = res.tile([128, KA, BLOC], F32, tag="mask")
            nc.sync.dma_start(out=mask_sb, in_=maskD.rearrange("(c p) b -> p c b", p=128))
            ident_sb = res.tile([128, 128], BF16, tag="ident")
            nc.sync.dma_start(out=ident_sb, in_=identD[:, :])
            brsv_sb = res.tile([128, KH], F32, tag="brsv")
            nc.sync.dma_start(out=brsv_sb, in_=biasRSV[:, :])
            bih_sb = res.tile([128, KM], F32, tag="bih")
            nc.sync.dma_start(out=bih_sb, in_=biasIH[:, :])
            bw_sb = res.tile([1, 1], F32, tag="bw")
            nc.sync.dma_start(out=bw_sb, in_=bw[:, :])
            # PS: proj_S + (b_S+b_R+b_V), [128, KH, BL] bf16
            ps_sb = res.tile([128, KH, BL], BF16, tag="ps")

            # ---------- precompute phase ----------
            with (
                tc.tile_pool(name="prew", bufs=1) as prew,
                tc.tile_pool(name="prehv", bufs=4) as prehv,
                tc.tile_pool(name="prestg", bufs=2) as prestg,
                tc.tile_pool(name="prepsum", bufs=4, space="PSUM") as prepsum,
            ):
                ws_sb = prew.tile([128, KH, H], BF16, tag="ws")
                nc.sync.dma_start(out=ws_sb, in_=WS_T.rearrange("(k p) m -> p k m", p=128))
                wv_sb = prew.tile([128, KH, H], BF16, tag="wv")
                nc.sync.dma_start(out=wv_sb, in_=WV_T.rearrange("(k p) m -> p k m", p=128))
                wihv_sb = prew.tile([128, KH, G4], BF16, tag="wihv")
                nc.sync.dma_start(out=wihv_sb, in_=WihV_T.rearrange("(k p) m -> p k m", p=128))
                wihs_sb = prew.tile([128, KH, G4], BF16, tag="wihs")
                nc.sync.dma_start(out=wihs_sb, in_=WihS_T.rearrange("(k p) m -> p k m", p=128))
                hsT_sb = prew.tile([128, KH, BL], BF16, tag="hsT")
                nc.sync.dma_start(out=hsT_sb, in_=hsT.rearrange("(k p) n -> p k n", p=128))

                # PS = W_S @ hsT + biasRSV
                for m in range(KH):
                    pps = prepsum.tile([128, BL], F32, tag="pp")
                    for kc in range(KH):
                        nc.tensor.matmul(
                            pps,
                            ws_sb[:, kc, m * 128:(m + 1) * 128],
                            hsT_sb[:, kc, :],
                            start=(kc == 0), stop=(kc == KH - 1),
                        )
                    nc.vector.tensor_scalar_add(ps_sb[:, m, :], pps, brsv_sb[:, m:m + 1])

                # PSg^T = h_s @ W_ihS^T in [(b,l), G4] layout: attention output
                # folded into the gate matmul (gates_S = PSg^T.T-style @ A).
                for c in range(KA):
                    for blk in range(KH):
                        ppg = prepsum.tile([128, 512], F32, tag="ppg")
                        for kc in range(KH):
                            nc.tensor.matmul(
                                ppg,
                                hsT_sb[:, kc, c * 128:(c + 1) * 128],
                                wihs_sb[:, kc, blk * 512:(blk + 1) * 512],
                                start=(kc == 0), stop=(kc == KH - 1),
                            )
                        nc.scalar.copy(psg_sb[:, c, blk * 512:(blk + 1) * 512], ppg)

                # PV / GV over hvT N-chunks
                for ncnk in range(n_nc):
                    nsl = slice(ncnk * NCW, (ncnk + 1) * NCW)
                    hv_t = []
                    for kc in range(KH):
                        t_ = prehv.tile([128, NCW], BF16, tag="hv")
                        nc.sync.dma_start(
                            out=t_, in_=hvT[kc * 128:(kc + 1) * 128, nsl])
                        hv_t.append(t_)
                    t0 = ncnk * NCW // BLOC
                    tw = NCW // BLOC
                    for m in range(KH):
                        ppv = prepsum.tile([128, NCW], F32, tag="pp")
                        for kc in range(KH):
                            nc.tensor.matmul(
                                ppv, wv_sb[:, kc, m * 128:(m + 1) * 128],
                                hv_t[kc], start=(kc == 0), stop=(kc == KH - 1))
                        stg = prestg.tile([128, NCW], BF16, tag="pvstg")
                        nc.scalar.copy(stg, ppv)
                        nc.sync.dma_start(
                            out=PV_d[t0:t0 + tw, m, :, :].rearrange("t p b -> p t b"),
                            in_=stg.rearrange("p (t b) -> p t b", b=BLOC))
                    for m in range(KM):
                        pgv = prepsum.tile([128, NCW], F32, tag="pp")
                        for kc in range(KH):
                            nc.tensor.matmul(
                                pgv, wihv_sb[:, kc, m * 128:(m + 1) * 128],
                                hv_t[kc], start=(kc == 0), stop=(kc == KH - 1))
                        stg = prestg.tile([128, NCW], BF16, tag="gvstg")
                        nc.vector.tensor_scalar_add(stg, pgv, bih_sb[:, m:m + 1])
                        nc.sync.dma_start(
                            out=GV_d[t0:t0 + tw, m, :, :].rearrange("t p b -> p t b"),
                            in_=stg.rearrange("p (t b) -> p t b", b=BLOC))

            # ---------- recurrence ----------
            psum = tc.alloc_tile_pool(name="psum", bufs=1, space="PSUM")
            czero = state.tile([128, 32], F32, tag="c")
            nc.vector.memset(czero, 0.0)
            hzero = res.tile([128, 32], BF16, tag="h0")
            nc.vector.memset(hzero, 0.0)
            c_prev = czero
            h_prev = hzero  # [128, (kc,b)] bf16

            HB = 8
            gv_cur = pv_cur = None
            hbuf = None

            for t in range(T):
                ic = t % Tc
                if ic == 0:
                    gv_cur = stream.tile([128, Tc, KM, BLOC], BF16, tag="gv")
                    nc.sync.dma_start(
                        out=gv_cur,
                        in_=GV_d[t:t + Tc].rearrange("t m p b -> p t m b"))
                    pv_cur = stream.tile([128, Tc, KH, BLOC], BF16, tag="pv")
                    nc.sync.dma_start(
                        out=pv_cur,
                        in_=PV_d[t:t + Tc].rearrange("t k p b -> p t k b"))
                ts_ = t % HB
                if ts_ == 0:
                    hbuf = houtp.tile([128, HB, KH, BLOC], BF16, tag="hb")

                # 1. proj_R + PV[t] -> psum_rv [128, (m,b)]; single PSUM group
                # (first matmul start=True; PV folded via identity matmul).
                psum_rv = psum.tile([128, KH, BLOC], F32, tag="rv")
                for m in range(KH):
                    for kc in range(KH):
                        nc.tensor.matmul(
                            psum_rv[:, m, :],
                            wr_sb[:, kc, m * 128:(m + 1) * 128],
                            h_prev[:, kc * BLOC:(kc + 1) * BLOC],
                            start=(m == 0 and kc == 0), stop=False,
                            skip_group_check=True)
                nc.tensor.matmul(
                    psum_rv.rearrange("p k b -> p (k b)"),
                    ident_sb, pv_cur[:, ic].rearrange("p k b -> p (k b)"),
                    start=False, stop=True, skip_group_check=True)
                # 2. gates: W_hh @ h_prev part first (overlaps attention chain).
                # Single PSUM accumulation group for W_hh + PSg + GV.
                psum_gh = psum.tile([128, KM, BLOC], F32, tag="gh")
                for m in range(KM):
                    for kc in range(KH):
                        nc.tensor.matmul(
                            psum_gh[:, m, :],
                            whh_sb[:, kc, m * 128:(m + 1) * 128],
                            h_prev[:, kc * BLOC:(kc + 1) * BLOC],
                            start=(m == 0 and kc == 0), stop=False,
                            skip_group_check=True)
                # 3. rvt = copy of psum_rv (bf16, for bcast reads)
                rvt = work.tile([128, KH * BLOC], BF16, tag="rvt")
                nc.vector.tensor_copy(
                    rvt.rearrange("p (k b) -> p k b", b=BLOC), psum_rv)
                # 4. e = tanh(PS + bcast_L(rvt)); (l,b) free layout so the
                # broadcast AP has stride-1 innermost -> DVE 2x mode.
                e_all = work.tile([128, KH, BL], BF16, tag="e")
                for pr in range(2):
                    sl = rvt[:, pr * 2 * BLOC:(pr * 2 + 2) * BLOC]
                    bc = bass.AP(tensor=sl.tensor, offset=sl.offset,
                                 ap=[sl.ap[0], [BLOC, 2], [0, L], [1, BLOC]])
                    nc.vector.tensor_tensor(
                        e_all[:, 2 * pr:2 * pr + 2].rearrange(
                            "p k (l b) -> p k l b", b=BLOC),
                        ps_sb[:, 2 * pr:2 * pr + 2].rearrange(
                            "p k (l b) -> p k l b", b=BLOC),
                        bc, ALU.add)
                    nc.scalar.activation(
                        e_all[:, 2 * pr:2 * pr + 2],
                        e_all[:, 2 * pr:2 * pr + 2], AF.Tanh)
                psum_beta = psum.tile([1, BL], F32, tag="beta")
                for kc in range(KH):
                    nc.tensor.matmul(
                        psum_beta, wvec_sb[:, kc:kc + 1], e_all[:, kc],
                        start=(kc == 0), stop=(kc == KH - 1))
                # 5. softmax pieces: expb (bf16), denom, rec
                expb = work.tile([1, BL], BF16, tag="expb")
                nc.scalar.activation(expb, psum_beta, AF.Exp, bias=bw_sb[:, 0:1])
                denom = work.tile([1, BLOC], F32, tag="denom")
                bexp = bass.AP(tensor=expb.tensor, offset=expb.offset,
                               ap=[expb.ap[0], [1, BLOC], [BLOC, L]])
                nc.vector.tensor_reduce(denom, bexp, AX.X, ALU.add)
                rec = work.tile([1, BLOC], F32, tag="rec")
                nc.vector.reciprocal(rec, denom)
                rec_bf = work.tile([1, BLOC], BF16, tag="recbf")
                nc.vector.tensor_copy(rec_bf, rec)
                # 6. alpha onto partitions: psum_A[c] = expb_chunk^T (x) rec
                psum_A = psum.tile([128, KA, BLOC], F32, tag="A")
                for c in range(KA):
                    nc.tensor.matmul(
                        psum_A[:, c, :],
                        expb[:, c * 128:(c + 1) * 128],
                        rec_bf, start=True, stop=True)
                A_sb = work.tile([128, KA, BLOC], BF16, tag="Asb")
                nc.vector.tensor_tensor(A_sb, psum_A, mask_sb, ALU.mult)
                # 7. gates_S = PSg^T @ A and GV[t] (identity matmul) accumulate
                # into the same PSUM group
                for m in range(KM):
                    for c in range(KA):
                        nc.tensor.matmul(
                            psum_gh[:, m, :],
                            psg_sb[:, c, m * 128:(m + 1) * 128],
                            A_sb[:, c, :],
                            start=False, stop=False,
                            skip_group_check=True)
                nc.tensor.matmul(
                    psum_gh.rearrange("p m b -> p (m b)"),
                    ident_sb, gv_cur[:, ic].rearrange("p m b -> p (m b)"),
                    start=False, stop=True, skip_group_check=True)
                # 8. pointwise from PSUM: th = tanh(gates) (i/f/o pre-scaled
                # 0.5); sigma = 0.5*th+0.5 via scalar Copy (same engine)
                th = work.tile([128, KM * BLOC], F32, tag="th")
                nc.scalar.activation(
                    th, psum_gh.rearrange("p m b -> p (m b)"), AF.Tanh)
                sio = work.tile([128, 96], F32, tag="sio")
                nc.scalar.activation(sio, th[:, 0:96], AF.Copy, bias=0.5, scale=0.5)
                # keepalive: tiny matmul pulses keep the PE HAM busy-window
                # alive through the pointwise tail (else every step restarts
                # at the 1.2 GHz throttled clock).
                scr = psum.tile([1, 1], F32, tag="scr")
                nc.tensor.matmul(scr, th[0:1, 0:1], th[0:1, 1:2],
                                 start=True, stop=True)
                t1 = work.tile([128, 32], F32, tag="t1")
                nc.vector.tensor_tensor(t1, sio[:, 0:32], th[:, 96:128], ALU.mult)
                t2 = work.tile([128, 32], F32, tag="t2")
                nc.vector.tensor_tensor(t2, sio[:, 32:64], c_prev, ALU.mult)
                c_new = state.tile([128, 32], F32, tag="c")
                nc.vector.tensor_tensor(c_new, t1, t2, ALU.add)
                tc_ = work.tile([128, 32], F32, tag="tc")
                nc.scalar.activation(tc_, c_new, AF.Tanh)
                scr2 = psum.tile([1, 1], F32, tag="scr2")
                nc.tensor.matmul(scr2, tc_[0:1, 0:1], tc_[0:1, 1:2],
                                 start=True, stop=True)
                h_new = hbuf[:, ts_]  # [128, KH, BLOC] bf16 view
                nc.vector.tensor_tensor(
                    h_new.rearrange("p k b -> p (k b)"), sio[:, 64:96], tc_, ALU.mult)
                if debug and t == dbg_t:
                    nc.sync.dma_start(out=dbg["dbg_rvt"][:, :], in_=rvt)
                    nc.sync.dma_start(out=dbg["dbg_e"][:, :, :], in_=e_all)
                    nc.sync.dma_start(out=dbg["dbg_expb"][:, :], in_=expb)
                    nc.sync.dma_start(out=dbg["dbg_A"][:, :, :], in_=A_sb)

                    nc.sync.dma_start(out=dbg["dbg_th"][:, :], in_=th)
                    nc.sync.dma_start(out=dbg["dbg_c"][:, :], in_=c_new)
                c_prev = c_new
                h_prev = h_new.rearrange("p k b -> p (k b)")
                # 11. flush h ring
                if ts_ == HB - 1 or t == T - 1:
                    nb = ts_ + 1
                    t0 = t - nb + 1
                    nc.sync.dma_start(
                        out=out_c[t0:t0 + nb].rearrange("t k p b -> p (t k) b"),
                        in_=hbuf[:, :nb].rearrange("p t k b -> p (t k) b"))
            psum.release()
    nc.finalize()
    return nc


# ---------------- host side ----------------

def _gate_remap():
    """Row permutation + scale for gate order [i, f, o, g], i/f/o scaled 0.5."""
    idx = np.concatenate([
        np.arange(0, 512), np.arange(512, 1024),
        np.arange(1536, 2048), np.arange(1024, 1536)])
    scale = np.concatenate([
        np.full(1536, 0.5, np.float32), np.ones(512, np.float32)])
    return idx, scale


def prep_core_inputs(h_v, h_s, W, T=T_FULL):
    """Per-core input maps. W: dict of full weight arrays."""
    idx, gsc = _gate_remap()
    W_ih2 = W["W_ih"][idx] * gsc[:, None]
    W_hh2 = W["W_hh"][idx] * gsc[:, None]
    b2 = (W["b_ih"] + W["b_hh"])[idx] * gsc

    WS_T = np.ascontiguousarray(W["W_S"].T).astype(NP_BF16)
    WV_T = np.ascontiguousarray(W["W_V"].T).astype(NP_BF16)
    WihV_T = np.ascontiguousarray(W_ih2[:, :DV].T).astype(NP_BF16)
    WihS_T = np.ascontiguousarray(W_ih2[:, DV:].T).astype(NP_BF16)
    Whh_T = np.ascontiguousarray(W_hh2.T).astype(NP_BF16)
    WR_T = np.ascontiguousarray(W["W_R"].T).astype(NP_BF16)
    wvec = np.ascontiguousarray(W["W_w"][0][:, None]).astype(NP_BF16)
    biasRSV = np.ascontiguousarray(
        (W["b_S"] + W["b_R"] + W["b_V"]).reshape(KH, 128).T).astype(np.float32)
    biasIH = np.ascontiguousarray(b2.reshape(KM, 128).T).astype(np.float32)
    bw = np.ascontiguousarray(W["b_w"].reshape(1, 1)).astype(np.float32)
    # (l, b) free-dim order: position j = l*BLOC + b
    maskD = np.zeros((BL, BLOC), np.float32)
    for j in range(BL):
        maskD[j, j % BLOC] = 1.0
    identD = np.eye(128, dtype=np.float32).astype(NP_BF16)
    maps = []
    for c in range(NCORES):
        bs = slice(c * BLOC, (c + 1) * BLOC)
        hvT = np.ascontiguousarray(
            h_v[bs, :T].transpose(2, 1, 0).reshape(DV, T * BLOC)).astype(NP_BF16)
        hsT = np.ascontiguousarray(
            h_s[bs].transpose(2, 1, 0).reshape(DS, L * BLOC)).astype(NP_BF16)
        maps.append({
            "hvT": hvT, "hsT": hsT,
            "WS_T": WS_T, "WV_T": WV_T,
            "WihV_T": WihV_T, "WihS_T": WihS_T, "Whh_T": Whh_T, "WR_T": WR_T,
            "wvec": wvec, "maskD": maskD, "identD": identD,
            "biasRSV": biasRSV, "biasIH": biasIH, "bw": bw,
        })
    return maps


_NC_CACHE = {}


def kernel(**inputs):
    h_v = np.asarray(inputs["h_v"], dtype=np.float32)
    h_s = np.asarray(inputs["h_s"], dtype=np.float32)
    W = {k: np.asarray(v, dtype=np.float32) for k, v in inputs.items()}
    key = "full"
    if key not in _NC_CACHE:
        _NC_CACHE[key] = build_nc(T=T_FULL, Tc=16)
    nc = _NC_CACHE[key]
    maps = prep_core_inputs(h_v, h_s, W, T=T_FULL)
    res = run_bass_kernel_spmd(nc, maps, list(range(NCORES)))
    outs = []
    for c in range(NCORES):
        arr = np.asarray(res.results[c]["out_c"]).astype(np.float32)
        outs.append(np.ascontiguousarray(
            arr.transpose(3, 0, 1, 2).reshape(BLOC, T_FULL, H)))
    return np.concatenate(outs, axis=0).astype(np.float32)


if __name__ == "__main__":
    nc = build_nc(T=8, Tc=4)
    print("built ok")


# revision 44
# speedup vs baseline: 1.5275x; 1.5275x over previous
"""Trainium2 Bass kernel for nn_Interactor (attention-augmented LSTM).

Problem: B=64, T=512, L=48, DV=DS=H=512.
  per step t: proj_V = x_t W_V^T; proj_R = h W_R^T
              e = tanh(proj_R[:,None,:] + proj_S + proj_V[:,None,:])
              alpha = softmax(e @ w, axis=L); h_ts = alpha @ h_s
              gates = [x_t, h_ts] W_ih^T + h W_hh^T + b; LSTM cell update.

Sharding: data-parallel over batch: 8 cores x 8 batch rows, weights replicated.

v2 design notes (vs v1):
 - All tensor-engine operands in bf16: fp32 matmuls are emitted as 2 HW
   instructions (HI/LO split) and disable fast-weight-load; bf16 halves
   instruction count and cuts LDWEIGHTS from ~325ns to ~55ns per tile.
 - LSTM sigmoid eliminated via sigma(x) = 0.5*tanh(x/2)+0.5 (i/f/o weight rows
   pre-scaled 0.5 on host). Kernel uses only {tanh, exp} => single ACT table
   set, killing 2x ACT_TABLE_LOAD (~2.7us) per step.
 - h_ts = alpha @ h_s computed on the tensor engine: alpha lands on
   partitions via outer-product matmul expb_chunk^T (x) rec, masked to
   block-diagonal A, then h_s-stationary matmuls produce h_ts^T [D,b].
 - W_hh @ h_prev issued right after proj_R so it overlaps the attention
   chain on DVE/ACT; W_ihS @ h_ts completes the PSUM group later.
 - Gate order remapped to [i, f, o, g] so one batched tanh + one affine
   covers the pointwise nonlinearities.
"""

import numpy as np
import ml_dtypes

import concourse.bass as bass
import concourse.mybir as mybir
import concourse.tile as tile
from concourse import bacc
from concourse.bass_utils import run_bass_kernel_spmd

F32 = mybir.dt.float32
BF16 = mybir.dt.bfloat16
AF = mybir.ActivationFunctionType
ALU = mybir.AluOpType
AX = mybir.AxisListType
NP_BF16 = ml_dtypes.bfloat16

B, T_FULL, L = 64, 512, 48
DV, DS, H = 512, 512, 512
G4 = 4 * H
NCORES = 8
BLOC = B // NCORES  # 8
BL = BLOC * L       # 384
KH = H // 128       # 4 H-chunks
KM = G4 // 128      # 16 gate-row chunks
KA = BL // 128      # 3 (b,l)-chunks


def build_nc(T=T_FULL, Tc=16, debug=False, dbg_t=0):
    """Build the per-core Bass program (SPMD; same program all cores)."""
    assert T % Tc == 0
    nc = bacc.Bacc()

    # ---- DRAM I/O (per-core slices fed via in_maps) ----
    hvT = nc.declare_dram_parameter("hvT", [DV, T * BLOC], BF16, isOutput=False)
    hsT = nc.declare_dram_parameter("hsT", [DS, BL], BF16, isOutput=False)
    WS_T = nc.declare_dram_parameter("WS_T", [DS, H], BF16, isOutput=False)
    WV_T = nc.declare_dram_parameter("WV_T", [DV, H], BF16, isOutput=False)
    WihV_T = nc.declare_dram_parameter("WihV_T", [DV, G4], BF16, isOutput=False)
    WihS_T = nc.declare_dram_parameter("WihS_T", [DS, G4], BF16, isOutput=False)
    Whh_T = nc.declare_dram_parameter("Whh_T", [H, G4], BF16, isOutput=False)
    WR_T = nc.declare_dram_parameter("WR_T", [H, H], BF16, isOutput=False)
    wvec = nc.declare_dram_parameter("wvec", [H, 1], BF16, isOutput=False)
    maskD = nc.declare_dram_parameter("maskD", [BL, BLOC], F32, isOutput=False)
    identD = nc.declare_dram_parameter("identD", [128, 128], BF16, isOutput=False)
    biasRSV = nc.declare_dram_parameter("biasRSV", [128, KH], F32, isOutput=False)
    biasIH = nc.declare_dram_parameter("biasIH", [128, KM], F32, isOutput=False)
    bw = nc.declare_dram_parameter("bw", [1, 1], F32, isOutput=False)
    out_c = nc.declare_dram_parameter("out_c", [KH, 128, T, BLOC], BF16, isOutput=True)

    # ---- internal DRAM for precomputed projections ----
    # m-major layouts: DMA stores/loads keep >=256B contiguous runs per
    # partition (16B-run transposing scatters measured ~13 GB/s).
    GV_d = nc.dram_tensor("GV_d", [KM, 128, T, BLOC], BF16)
    PV_d = nc.dram_tensor("PV_d", [KH, 128, T, BLOC], BF16)

    if debug:
        dbg = {
            "dbg_rvt": nc.dram_tensor("dbg_rvt", [128, KH * BLOC], BF16, kind="ExternalOutput"),
            "dbg_e": nc.dram_tensor("dbg_e", [128, KH, BL], BF16, kind="ExternalOutput"),
            "dbg_expb": nc.dram_tensor("dbg_expb", [1, BL], BF16, kind="ExternalOutput"),
            "dbg_A": nc.dram_tensor("dbg_A", [128, KA, BLOC], BF16, kind="ExternalOutput"),
            "dbg_gates": nc.dram_tensor("dbg_gates", [128, KM * BLOC], F32, kind="ExternalOutput"),
            "dbg_th": nc.dram_tensor("dbg_th", [128, KM * BLOC], F32, kind="ExternalOutput"),
            "dbg_c": nc.dram_tensor("dbg_c", [128, 32], F32, kind="ExternalOutput"),
        }

    NT = T * BLOC
    NCW = min(512, NT)
    n_nc = NT // NCW

    with tile.TileContext(nc) as tc:
        with (
            tc.tile_pool(name="res", bufs=1) as res,
            tc.tile_pool(name="state", bufs=2) as state,
            tc.tile_pool(name="hout", bufs=2) as houtp,
            tc.tile_pool(name="stream", bufs=2) as stream,
            tc.tile_pool(name="work", bufs=2) as work,
        ):
            # ---------- resident loads ----------
            wr_sb = res.tile([128, KH, H], BF16, tag="wr")
            nc.sync.dma_start(out=wr_sb, in_=WR_T.rearrange("(k p) m -> p k m", p=128))
            whh_sb = res.tile([128, KH, G4], BF16, tag="whh")
            nc.sync.dma_start(out=whh_sb, in_=Whh_T.rearrange("(k p) m -> p k m", p=128))
            psg_sb = res.tile([128, KA, G4], BF16, tag="psg")
            wvec_sb = res.tile([128, KH], BF16, tag="wvec")
            nc.sync.dma_start(out=wvec_sb, in_=wvec.rearrange("(k p) o -> p (k o)", p=128))
            mask_sb = res.tile([128, KA, BLOC], F32, tag="mask")
            nc.sync.dma_start(out=mask_sb, in_=maskD.rearrange("(c p) b -> p c b", p=128))
            ident_sb = res.tile([128, 128], BF16, tag="ident")
            nc.sync.dma_start(out=ident_sb, in_=identD[:, :])
            brsv_sb = res.tile([128, KH], F32, tag="brsv")
            nc.sync.dma_start(out=brsv_sb, in_=biasRSV[:, :])
            bih_sb = res.tile([128, KM], F32, tag="bih")
            nc.sync.dma_start(out=bih_sb, in_=biasIH[:, :])
            bw_sb = res.tile([1, 1], F32, tag="bw")
            nc.sync.dma_start(out=bw_sb, in_=bw[:, :])
            # PS: proj_S + (b_S+b_R+b_V), [128, KH, BL] bf16
            ps_sb = res.tile([128, KH, BL], BF16, tag="ps")

            # ---------- precompute phase ----------
            with (
                tc.tile_pool(name="prew", bufs=1) as prew,
                tc.tile_pool(name="prehv", bufs=4) as prehv,
                tc.tile_pool(name="prestg", bufs=2) as prestg,
                tc.tile_pool(name="prepsum", bufs=4, space="PSUM") as prepsum,
            ):
                ws_sb = prew.tile([128, KH, H], BF16, tag="ws")
                nc.sync.dma_start(out=ws_sb, in_=WS_T.rearrange("(k p) m -> p k m", p=128))
                wv_sb = prew.tile([128, KH, H], BF16, tag="wv")
                nc.sync.dma_start(out=wv_sb, in_=WV_T.rearrange("(k p) m -> p k m", p=128))
                wihv_sb = prew.tile([128, KH, G4], BF16, tag="wihv")
                nc.sync.dma_start(out=wihv_sb, in_=WihV_T.rearrange("(k p) m -> p k m", p=128))
                wihs_sb = prew.tile([128, KH, G4], BF16, tag="wihs")
                nc.sync.dma_start(out=wihs_sb, in_=WihS_T.rearrange("(k p) m -> p k m", p=128))
                hsT_sb = prew.tile([128, KH, BL], BF16, tag="hsT")
                nc.sync.dma_start(out=hsT_sb, in_=hsT.rearrange("(k p) n -> p k n", p=128))

                # PS = W_S @ hsT + biasRSV
                for m in range(KH):
                    pps = prepsum.tile([128, BL], F32, tag="pp")
                    for kc in range(KH):
                        nc.tensor.matmul(
                            pps,
                            ws_sb[:, kc, m * 128:(m + 1) * 128],
                            hsT_sb[:, kc, :],
                            start=(kc == 0), stop=(kc == KH - 1),
                        )
                    nc.vector.tensor_scalar_add(ps_sb[:, m, :], pps, brsv_sb[:, m:m + 1])

                # PSg^T = h_s @ W_ihS^T in [(b,l), G4] layout: attention output
                # folded into the gate matmul (gates_S = PSg^T.T-style @ A).
                for c in range(KA):
                    for blk in range(KH):
                        ppg = prepsum.tile([128, 512], F32, tag="ppg")
                        for kc in range(KH):
                            nc.tensor.matmul(
                                ppg,
                                hsT_sb[:, kc, c * 128:(c + 1) * 128],
                                wihs_sb[:, kc, blk * 512:(blk + 1) * 512],
                                start=(kc == 0), stop=(kc == KH - 1),
                            )
                        nc.scalar.copy(psg_sb[:, c, blk * 512:(blk + 1) * 512], ppg)

                # PV / GV over hvT N-chunks
                for ncnk in range(n_nc):
                    nsl = slice(ncnk * NCW, (ncnk + 1) * NCW)
                    hv_t = []
                    for kc in range(KH):
                        t_ = prehv.tile([128, NCW], BF16, tag="hv")
                        nc.sync.dma_start(
                            out=t_, in_=hvT[kc * 128:(kc + 1) * 128, nsl])
                        hv_t.append(t_)
                    t0 = ncnk * NCW // BLOC
                    tw = NCW // BLOC
                    for m in range(KH):
                        ppv = prepsum.tile([128, NCW], F32, tag="pp")
                        for kc in range(KH):
                            nc.tensor.matmul(
                                ppv, wv_sb[:, kc, m * 128:(m + 1) * 128],
                                hv_t[kc], start=(kc == 0), stop=(kc == KH - 1))
                        stg = prestg.tile([128, NCW], BF16, tag="pvstg")
                        nc.scalar.copy(stg, ppv)
                        nc.sync.dma_start(
                            out=PV_d[m, :, t0:t0 + tw, :],
                            in_=stg.rearrange("p (t b) -> p t b", b=BLOC))
                    for m in range(KM):
                        pgv = prepsum.tile([128, NCW], F32, tag="pp")
                        for kc in range(KH):
                            nc.tensor.matmul(
                                pgv, wihv_sb[:, kc, m * 128:(m + 1) * 128],
                                hv_t[kc], start=(kc == 0), stop=(kc == KH - 1))
                        stg = prestg.tile([128, NCW], BF16, tag="gvstg")
                        nc.vector.tensor_scalar_add(stg, pgv, bih_sb[:, m:m + 1])
                        nc.sync.dma_start(
                            out=GV_d[m, :, t0:t0 + tw, :],
                            in_=stg.rearrange("p (t b) -> p t b", b=BLOC))

            # ---------- recurrence ----------
            psum = tc.alloc_tile_pool(name="psum", bufs=1, space="PSUM")
            czero = state.tile([128, 32], F32, tag="c")
            nc.vector.memset(czero, 0.0)
            hzero = res.tile([128, 32], BF16, tag="h0")
            nc.vector.memset(hzero, 0.0)
            c_prev = czero
            h_prev = hzero  # [128, (kc,b)] bf16

            HB = 8
            hbuf = None
            n_chunks = T // Tc

            def load_chunk(j):
                g = stream.tile([128, KM, Tc, BLOC], BF16, tag="gv")
                nc.sync.dma_start(
                    out=g, in_=GV_d[:, :, j * Tc:(j + 1) * Tc, :].rearrange(
                        "m p t b -> p m t b"))
                p = stream.tile([128, KH, Tc, BLOC], BF16, tag="pv")
                nc.sync.dma_start(
                    out=p, in_=PV_d[:, :, j * Tc:(j + 1) * Tc, :].rearrange(
                        "k p t b -> p k t b"))
                return g, p

            nxt = load_chunk(0)
            gv_cur = pv_cur = None

            for t in range(T):
                ic = t % Tc
                if ic == 0:
                    gv_cur, pv_cur = nxt
                    j = t // Tc
                    if j + 1 < n_chunks:
                        nxt = load_chunk(j + 1)
                ts_ = t % HB
                if ts_ == 0:
                    hbuf = houtp.tile([128, HB, KH, BLOC], BF16, tag="hb")

                # 1. proj_R + PV[t] -> psum_rv [128, (m,b)]; single PSUM group
                # (first matmul start=True; PV folded via identity matmul).
                psum_rv = psum.tile([128, KH, BLOC], F32, tag="rv")
                for m in range(KH):
                    for kc in range(KH):
                        nc.tensor.matmul(
                            psum_rv[:, m, :],
                            wr_sb[:, kc, m * 128:(m + 1) * 128],
                            h_prev[:, kc * BLOC:(kc + 1) * BLOC],
                            start=(m == 0 and kc == 0), stop=False,
                            skip_group_check=True)
                nc.tensor.matmul(
                    psum_rv, ident_sb, pv_cur[:, :, ic, :],
                    start=False, stop=True, skip_group_check=True)
                # 2. gates: W_hh @ h_prev part first (overlaps attention chain).
                # Single PSUM accumulation group for W_hh + PSg + GV.
                psum_gh = psum.tile([128, KM, BLOC], F32, tag="gh")
                for m in range(KM):
                    for kc in range(KH):
                        nc.tensor.matmul(
                            psum_gh[:, m, :],
                            whh_sb[:, kc, m * 128:(m + 1) * 128],
                            h_prev[:, kc * BLOC:(kc + 1) * BLOC],
                            start=(m == 0 and kc == 0), stop=False,
                            skip_group_check=True)
                # 3. rvt = copy of psum_rv (bf16, for bcast reads)
                rvt = work.tile([128, KH * BLOC], BF16, tag="rvt")
                nc.vector.tensor_copy(
                    rvt.rearrange("p (k b) -> p k b", b=BLOC), psum_rv)
                # 4. e = tanh(PS + bcast_L(rvt)); (l,b) free layout so the
                # broadcast AP has stride-1 innermost -> DVE 2x mode.
                e_all = work.tile([128, KH, BL], BF16, tag="e")
                for pr in range(2):
                    sl = rvt[:, pr * 2 * BLOC:(pr * 2 + 2) * BLOC]
                    bc = bass.AP(tensor=sl.tensor, offset=sl.offset,
                                 ap=[sl.ap[0], [BLOC, 2], [0, L], [1, BLOC]])
                    nc.vector.tensor_tensor(
                        e_all[:, 2 * pr:2 * pr + 2].rearrange(
                            "p k (l b) -> p k l b", b=BLOC),
                        ps_sb[:, 2 * pr:2 * pr + 2].rearrange(
                            "p k (l b) -> p k l b", b=BLOC),
                        bc, ALU.add)
                    nc.scalar.activation(
                        e_all[:, 2 * pr:2 * pr + 2],
                        e_all[:, 2 * pr:2 * pr + 2], AF.Tanh)
                psum_beta = psum.tile([1, BL], F32, tag="beta")
                for kc in range(KH):
                    nc.tensor.matmul(
                        psum_beta, wvec_sb[:, kc:kc + 1], e_all[:, kc],
                        start=(kc == 0), stop=(kc == KH - 1))
                # 5. softmax pieces: expb (bf16), denom, rec
                expb = work.tile([1, BL], BF16, tag="expb")
                nc.scalar.activation(expb, psum_beta, AF.Exp, bias=bw_sb[:, 0:1])
                denom = work.tile([1, BLOC], F32, tag="denom")
                bexp = bass.AP(tensor=expb.tensor, offset=expb.offset,
                               ap=[expb.ap[0], [1, BLOC], [BLOC, L]])
                nc.vector.tensor_reduce(denom, bexp, AX.X, ALU.add)
                rec = work.tile([1, BLOC], F32, tag="rec")
                nc.vector.reciprocal(rec, denom)
                rec_bf = work.tile([1, BLOC], BF16, tag="recbf")
                nc.vector.tensor_copy(rec_bf, rec)
                # 6. alpha onto partitions: psum_A[c] = expb_chunk^T (x) rec
                psum_A = psum.tile([128, KA, BLOC], F32, tag="A")
                for c in range(KA):
                    nc.tensor.matmul(
                        psum_A[:, c, :],
                        expb[:, c * 128:(c + 1) * 128],
                        rec_bf, start=True, stop=True)
                A_sb = work.tile([128, KA, BLOC], BF16, tag="Asb")
                nc.vector.tensor_tensor(A_sb, psum_A, mask_sb, ALU.mult)
                # 7. gates_S = PSg^T @ A and GV[t] (identity matmul) accumulate
                # into the same PSUM group
                for m in range(KM):
                    for c in range(KA):
                        nc.tensor.matmul(
                            psum_gh[:, m, :],
                            psg_sb[:, c, m * 128:(m + 1) * 128],
                            A_sb[:, c, :],
                            start=False, stop=False,
                            skip_group_check=True)
                nc.tensor.matmul(
                    psum_gh, ident_sb, gv_cur[:, :, ic, :],
                    start=False, stop=True, skip_group_check=True)
                # 8. pointwise from PSUM: th = tanh(gates) (i/f/o pre-scaled
                # 0.5); sigma = 0.5*th+0.5 via scalar Copy (same engine)
                th = work.tile([128, KM * BLOC], F32, tag="th")
                nc.scalar.activation(
                    th, psum_gh.rearrange("p m b -> p (m b)"), AF.Tanh)
                sio = work.tile([128, 96], F32, tag="sio")
                nc.scalar.activation(sio, th[:, 0:96], AF.Copy, bias=0.5, scale=0.5)
                # keepalive: tiny matmul pulses keep the PE HAM busy-window
                # alive through the pointwise tail (else every step restarts
                # at the 1.2 GHz throttled clock).
                scr = psum.tile([1, 1], F32, tag="scr")
                nc.tensor.matmul(scr, th[0:1, 0:1], th[0:1, 1:2],
                                 start=True, stop=True)
                t1 = work.tile([128, 32], F32, tag="t1")
                nc.vector.tensor_tensor(t1, sio[:, 0:32], th[:, 96:128], ALU.mult)
                t2 = work.tile([128, 32], F32, tag="t2")
                nc.vector.tensor_tensor(t2, sio[:, 32:64], c_prev, ALU.mult)
                c_new = state.tile([128, 32], F32, tag="c")
                nc.vector.tensor_tensor(c_new, t1, t2, ALU.add)
                tc_ = work.tile([128, 32], F32, tag="tc")
                nc.scalar.activation(tc_, c_new, AF.Tanh)
                scr2 = psum.tile([1, 1], F32, tag="scr2")
                nc.tensor.matmul(scr2, tc_[0:1, 0:1], tc_[0:1, 1:2],
                                 start=True, stop=True)
                h_new = hbuf[:, ts_]  # [128, KH, BLOC] bf16 view
                nc.vector.tensor_tensor(
                    h_new.rearrange("p k b -> p (k b)"), sio[:, 64:96], tc_, ALU.mult)
                if debug and t == dbg_t:
                    nc.sync.dma_start(out=dbg["dbg_rvt"][:, :], in_=rvt)
                    nc.sync.dma_start(out=dbg["dbg_e"][:, :, :], in_=e_all)
                    nc.sync.dma_start(out=dbg["dbg_expb"][:, :], in_=expb)
                    nc.sync.dma_start(out=dbg["dbg_A"][:, :, :], in_=A_sb)

                    nc.sync.dma_start(out=dbg["dbg_th"][:, :], in_=th)
                    nc.sync.dma_start(out=dbg["dbg_c"][:, :], in_=c_new)
                c_prev = c_new
                h_prev = h_new.rearrange("p k b -> p (k b)")
                # 11. flush h ring
                if ts_ == HB - 1 or t == T - 1:
                    nb = ts_ + 1
                    t0 = t - nb + 1
                    for k in range(KH):
                        nc.sync.dma_start(
                            out=out_c[k, :, t0:t0 + nb, :],
                            in_=hbuf[:, :nb, k, :])
            psum.release()
    nc.finalize()
    return nc


# ---------------- host side ----------------

def _gate_remap():
    """Row permutation + scale for gate order [i, f, o, g], i/f/o scaled 0.5."""
    idx = np.concatenate([
        np.arange(0, 512), np.arange(512, 1024),
        np.arange(1536, 2048), np.arange(1024, 1536)])
    scale = np.concatenate([
        np.full(1536, 0.5, np.float32), np.ones(512, np.float32)])
    return idx, scale


def prep_core_inputs(h_v, h_s, W, T=T_FULL):
    """Per-core input maps. W: dict of full weight arrays."""
    idx, gsc = _gate_remap()
    W_ih2 = W["W_ih"][idx] * gsc[:, None]
    W_hh2 = W["W_hh"][idx] * gsc[:, None]
    b2 = (W["b_ih"] + W["b_hh"])[idx] * gsc

    WS_T = np.ascontiguousarray(W["W_S"].T).astype(NP_BF16)
    WV_T = np.ascontiguousarray(W["W_V"].T).astype(NP_BF16)
    WihV_T = np.ascontiguousarray(W_ih2[:, :DV].T).astype(NP_BF16)
    WihS_T = np.ascontiguousarray(W_ih2[:, DV:].T).astype(NP_BF16)
    Whh_T = np.ascontiguousarray(W_hh2.T).astype(NP_BF16)
    WR_T = np.ascontiguousarray(W["W_R"].T).astype(NP_BF16)
    wvec = np.ascontiguousarray(W["W_w"][0][:, None]).astype(NP_BF16)
    biasRSV = np.ascontiguousarray(
        (W["b_S"] + W["b_R"] + W["b_V"]).reshape(KH, 128).T).astype(np.float32)
    biasIH = np.ascontiguousarray(b2.reshape(KM, 128).T).astype(np.float32)
    bw = np.ascontiguousarray(W["b_w"].reshape(1, 1)).astype(np.float32)
    # (l, b) free-dim order: position j = l*BLOC + b
    maskD = np.zeros((BL, BLOC), np.float32)
    for j in range(BL):
        maskD[j, j % BLOC] = 1.0
    identD = np.eye(128, dtype=np.float32).astype(NP_BF16)
    maps = []
    for c in range(NCORES):
        bs = slice(c * BLOC, (c + 1) * BLOC)
        hvT = np.ascontiguousarray(
            h_v[bs, :T].transpose(2, 1, 0).reshape(DV, T * BLOC)).astype(NP_BF16)
        hsT = np.ascontiguousarray(
            h_s[bs].transpose(2, 1, 0).reshape(DS, L * BLOC)).astype(NP_BF16)
        maps.append({
            "hvT": hvT, "hsT": hsT,
            "WS_T": WS_T, "WV_T": WV_T,
            "WihV_T": WihV_T, "WihS_T": WihS_T, "Whh_T": Whh_T, "WR_T": WR_T,
            "wvec": wvec, "maskD": maskD, "identD": identD,
            "biasRSV": biasRSV, "biasIH": biasIH, "bw": bw,
        })
    return maps


_NC_CACHE = {}


def kernel(**inputs):
    h_v = np.asarray(inputs["h_v"], dtype=np.float32)
    h_s = np.asarray(inputs["h_s"], dtype=np.float32)
    W = {k: np.asarray(v, dtype=np.float32) for k, v in inputs.items()}
    key = "full"
    if key not in _NC_CACHE:
        _NC_CACHE[key] = build_nc(T=T_FULL, Tc=16)
    nc = _NC_CACHE[key]
    maps = prep_core_inputs(h_v, h_s, W, T=T_FULL)
    res = run_bass_kernel_spmd(nc, maps, list(range(NCORES)))
    outs = []
    for c in range(NCORES):
        arr = np.asarray(res.results[c]["out_c"]).astype(np.float32)
        # [KH, 128, T, BLOC] -> [BLOC, T, KH*128]
        outs.append(np.ascontiguousarray(
            arr.transpose(3, 2, 0, 1).reshape(BLOC, T_FULL, H)))
    return np.concatenate(outs, axis=0).astype(np.float32)


if __name__ == "__main__":
    nc = build_nc(T=8, Tc=4)
    print("built ok")
